# revision 42
# baseline (speedup 1.0000x reference)
"""Causal attention kernel for Trainium2, 8 NeuronCores, sequence-parallel.

Reference computation (T=4096, D=1024, fp32):
    q = x @ Wqk; logits = q @ x.T (causal masked); attn = softmax(logits)
    out = (attn @ x) @ Wov

Causal load balancing under one SPMD program: the 32 query row-tiles of 128
are assigned to cores as {c, 15-c, 16+c, 31-c} and host-permuted into 4
local "slots" ordered by visibility class. Slot m processes a fixed key
budget of 8*(m+1) key-tiles (keys in natural order, prefix [0, 1024*(m+1))),
which covers every core's visible range in that class. Causality inside the
budget is enforced by a host-provided additive mask (0 / -60000) that also
carries the diagonal triangle, so the program is core-independent while
skipping 37.5% of the score/AV matmul work.

Matmul precision: fp16 inputs (x, Wqk, Wov, attn) with fp32 PSUM
accumulation; q and o1 kept in fp16 on-chip. Softmax row max subtracted in
fp32; attn stored fp16 for the DMA-xbar transposes and AV.

Scheduling notes: input DMAs are issued in consumption order (xqt/wqk for
phase A first, then keys/masks); phase B runs slots largest-first so the
exp/transpose pipeline drains during B and phase E can start right after;
tiles are split per dependency unit (per-kg keys, per-chunk attn-transpose,
per-d o1) to keep cross-engine waits granular.
"""

import sys

sys.path.insert(0, "/opt/trn_rl_repo")

import numpy as np

import concourse.tile as tile
from concourse import bacc, mybir
from concourse.bass_utils import run_bass_kernel_spmd

T = 4096
D = 1024
NCORES = 8
RQ = T // NCORES  # 512 query rows per core
KC = D // 128  # 8 contraction chunks
NEG16 = -57344.0  # exactly representable in fp8e5m2

BKT = [8, 16, 24, 32]  # key tiles (128) processed per slot
BG = [b // 4 for b in BKT]  # 512-wide key groups per slot
OFFK = [0, 1024, 3072, 6144]  # slot column offsets in ragged score layout
STOT = 10240  # total score/mask columns
MPOFF = [0, 2, 6, 12]  # mpart offsets (prefix of BG)
NCH = [b // 8 for b in BKT]  # 1024-wide exp chunks per slot: 1,2,3,4
LQOFF = [0, 1, 3, 6]  # lq offsets (prefix of NCH)

f32 = mybir.dt.float32
f16 = mybir.dt.float16
f8 = mybir.dt.float8e5


def _build_nc():
    nc = bacc.Bacc(
        "TRN2", target_bir_lowering=False, debug=False, num_devices=NCORES
    )

    xqt_d = nc.dram_tensor("xqt", [D, RQ], f16, kind="ExternalInput").ap()
    xtp_d = nc.dram_tensor("xtp", [D, T], f16, kind="ExternalInput").ap()
    xp_d = nc.dram_tensor("xp", [T, D], f16, kind="ExternalInput").ap()
    wqk_d = nc.dram_tensor("wqk", [D, D], f16, kind="ExternalInput").ap()
    wov_d = nc.dram_tensor("wov", [D, D], f16, kind="ExternalInput").ap()
    mask_d = nc.dram_tensor("mask", [128, STOT], f8, kind="ExternalInput").ap()
    ident_d = nc.dram_tensor("ident", [128, 128], f16, kind="ExternalInput").ap()
    out_d = nc.dram_tensor("out", [RQ, D], f32, kind="ExternalOutput").ap()

    with tile.TileContext(nc) as tc:
        # stack allocator: long-lived pools first
        consts = tc.alloc_tile_pool(name="consts", bufs=1)
        pt_pool = tc.alloc_tile_pool(name="ptpool", bufs=1)
        o1_pool = tc.alloc_tile_pool(name="o1pool", bufs=1)
        xpstream = tc.alloc_tile_pool(name="xpstream", bufs=3)
        p_pool = tc.alloc_tile_pool(name="ppool", bufs=2)
        s_pool = tc.alloc_tile_pool(name="spool", bufs=2)
        qt_pool = tc.alloc_tile_pool(name="qt", bufs=1)
        xtp_pool = tc.alloc_tile_pool(name="xtpp", bufs=1)
        mask_pool = tc.alloc_tile_pool(name="maskp", bufs=1)
        wqk_pool = tc.alloc_tile_pool(name="wqkp", bufs=1)
        xqt_pool = tc.alloc_tile_pool(name="xqtp", bufs=1)

        # stats scratch: negmax 0:4, lsum 4:8, recip 8:12, mpart 12:32, lq 32:42
        smalls = consts.tile([128, 48], f32, name="smalls")
        dum = consts.tile([128, 256], f16, name="dum")
        ident = consts.tile([128, 128], f16, name="ident")
        negmax = smalls[:, 0:4]
        lsum = smalls[:, 4:8]
        recip = smalls[:, 8:12]
        mpart = smalls[:, 12:32]
        lq = smalls[:, 32:42]

        # transposed attn in a ragged layout: for key-tile kt the active
        # slots' columns are contiguous, so phase E runs one matmul per
        # (kt, d). kt block c=kt//8 has width (4-c)*128 and holds slots
        # m >= c at relative position (m-c)*128.
        PTOFF = [0, 4096, 7168, 9216]  # prefix sums of 8*width(c)
        ptall = pt_pool.tile([128, STOT], f16, name="ptall")

        def pt_view(m, c):
            # [128, 8 kts, 128 rows] view of slot m's chunk c region
            width = (4 - c) * 128
            base = PTOFF[c] + (m - c) * 128
            return ptall[:, base : base + 8 * width].rearrange(
                "p (kt w) -> p kt w", kt=8
            )[:, :, 0:128]
        o1t = [o1_pool.tile([128, RQ], f16, name=f"o1t{d}") for d in range(KC)]
        qt_sb = qt_pool.tile([128, KC * RQ], f16, name="qt_sb")
        xtp_t = [
            xtp_pool.tile([128, KC * 512], f16, name=f"xtp{kg}")
            for kg in range(T // 512)
        ]
        mask_t = [
            mask_pool.tile([128, BKT[m] * 128], f8, name=f"mask{m}")
            for m in range(4)
        ]
        wqk_t = [
            wqk_pool.tile([128, KC * 256], f16, name=f"wqk{md2}")
            for md2 in range(KC // 2)
        ]
        xqt_sb = xqt_pool.tile([128, KC * RQ], f16, name="xqt_sb")

        # ---- input DMAs, issued in consumption order ---------------------
        def load_wqk(md2):
            nc.sync.dma_start(
                wqk_t[md2].rearrange("p (kc n) -> p kc n", kc=KC),
                wqk_d[:, md2 * 256 : (md2 + 1) * 256].rearrange(
                    "(kc p) n -> p kc n", p=128
                ),
            )

        def load_xtp(kg):
            nc.sync.dma_start(
                xtp_t[kg].rearrange("p (kc n) -> p kc n", kc=KC),
                xtp_d[:, kg * 512 : (kg + 1) * 512].rearrange(
                    "(kc p) n -> p kc n", p=128
                ),
            )

        def load_mask(m):
            nc.sync.dma_start(
                mask_t[m], mask_d[:, OFFK[m] : OFFK[m] + BKT[m] * 128]
            )

        nc.sync.dma_start(
            xqt_sb.rearrange("p (kc n) -> p kc n", kc=KC),
            xqt_d.rearrange("(kc p) n -> p kc n", p=128),
        )
        load_wqk(0)
        load_wqk(1)
        load_wqk(2)
        load_xtp(0)
        load_wqk(3)
        load_xtp(1)
        load_mask(3)
        for kg in range(2, 8):
            load_xtp(kg)
        load_mask(2)
        load_mask(1)
        load_mask(0)
        nc.sync.dma_start(ident, ident_d)

        # PE p-state warmup: the tensor engine downclocks when idle and
        # takes ~3us to re-ramp. Keep it hot with throwaway matmuls into a
        # dedicated PSUM bank while input DMAs land / cross-engine deps
        # resolve. psW is allocated first so its WAR chains stay PE-internal.
        psW = tc.alloc_tile_pool(name="psW", bufs=1, space="PSUM")
        wps = psW.tile([128, 512], f32, name="wps")
        psT = tc.alloc_tile_pool(name="psT", bufs=1, space="PSUM")
        psT1 = psT.tile([128, 2048], f16, name="psT1")
        psT0 = psT.tile([128, 1024], f16, name="psT0")
        nc.gpsimd.memset(dum[:], 0.0)

        def warm(n):
            for _ in range(n):
                nc.tensor.matmul(
                    wps[:, 0:256], dum[:, 0:128], dum[:], start=True, stop=True
                )

        warm(18)

        # ---- Phase A: qT = (xq @ Wqk)^T  -> [D, RQ] fp16 -----------------
        with tc.tile_pool(name="psA", bufs=2, space="PSUM") as psA:
            for md2 in range(KC // 2):
                for h in range(2):
                    mtd = md2 * 2 + h
                    ps = psA.tile([128, RQ], f32, name="ps_qt")
                    for kc in range(KC):
                        nc.tensor.matmul(
                            ps[:],
                            wqk_t[md2][
                                :, kc * 256 + h * 128 : kc * 256 + h * 128 + 128
                            ],
                            xqt_sb[:, kc * RQ : (kc + 1) * RQ],
                            start=(kc == 0),
                            stop=(kc == KC - 1),
                        )
                    nc.vector.tensor_copy(
                        qt_sb[:, mtd * RQ : (mtd + 1) * RQ], ps[:]
                    )
        xqt_pool.release()
        wqk_pool.release()

        # ---- Phase B: per-slot scores + mask + softmax stats + exp/T -----
        # slots largest-first so the exp/transpose pipeline overlaps B
        warm(4)
        p_q_late = {}
        with tc.tile_pool(name="psB", bufs=4, space="PSUM") as psB:
            for m in (3, 2, 1, 0):
                s_t = s_pool.tile([128, BKT[m] * 128], f32, name=f"s{m}", tag="s")
                for kg in range(BG[m]):
                    ps = psB.tile([128, 512], f32, name="ps_s", tag="psb")
                    for kc in range(KC):
                        nc.tensor.matmul(
                            ps[:],
                            qt_sb[:, kc * RQ + m * 128 : kc * RQ + (m + 1) * 128],
                            xtp_t[kg][:, kc * 512 : (kc + 1) * 512],
                            start=(kc == 0),
                            stop=(kc == KC - 1),
                        )
                    dst = s_t[:, kg * 512 : (kg + 1) * 512]
                    nc.vector.tensor_add(
                        dst, ps[:], mask_t[m][:, kg * 512 : (kg + 1) * 512]
                    )
                    nc.vector.tensor_reduce(
                        mpart[:, MPOFF[m] + kg : MPOFF[m] + kg + 1],
                        dst,
                        axis=mybir.AxisListType.X,
                        op=mybir.AluOpType.max,
                    )
                nc.vector.tensor_reduce(
                    negmax[:, m : m + 1],
                    mpart[:, MPOFF[m] : MPOFF[m] + BG[m]],
                    axis=mybir.AxisListType.X,
                    op=mybir.AluOpType.max,
                    negate=True,
                )
                # one exp + one transpose per slot keeps the Act queue
                # short (transpose issue blocks the Act sequencer ~1.3us).
                # Slots 1/0 finish after the xp-prefetch flood occupies the
                # DMA engines, so their transposes run on the PE (below)
                # instead of the xbar ring.
                p_q = p_pool.tile(
                    [128, BKT[m] * 128], f16, name="p_q", tag="pq"
                )
                nc.scalar.activation(
                    p_q[:],
                    s_t[:],
                    mybir.ActivationFunctionType.Exp,
                    bias=negmax[:, m : m + 1],
                    scale=1.0,
                    accum_out=lsum[:, m : m + 1],
                )
                if m >= 2:
                    for c in range(NCH[m]):
                        nc.scalar.dma_start_transpose(
                            pt_view(m, c),
                            p_q[:, c * 1024 : (c + 1) * 1024],
                        )
                else:
                    p_q_late[m] = p_q

        for m in range(4):
            nc.vector.reciprocal(recip[:, m : m + 1], lsum[:, m : m + 1])

        mask_pool.release()
        xtp_pool.release()
        qt_pool.release()
        s_pool.release()

        # slots 1/0: transpose attn on the PE (matmul vs identity) into
        # scratch PSUM, evacuated by the otherwise-idle DVE. psT banks are
        # fp16, 1024 cols per bank; start zeroes a whole bank.
        def pe_transpose(m):
            pst = psT1 if m == 1 else psT0
            for kt in range(BKT[m]):
                nc.tensor.matmul(
                    pst[:, kt * 128 : (kt + 1) * 128],
                    p_q_late[m][:, kt * 128 : (kt + 1) * 128],
                    ident,
                    is_transpose=True,
                    start=(kt % 8 == 0),
                    stop=(kt % 8 == 7),
                    skip_group_check=True,
                )
            for c in range(NCH[m]):
                nc.vector.tensor_copy(
                    pt_view(m, c),
                    pst[:, c * 1024 : (c + 1) * 1024].rearrange(
                        "p (kt r) -> p kt r", kt=8
                    ),
                )

        pe_transpose(1)
        # bridge slot0's exp latency and the psum-bank WAR with warmup
        # matmuls so E starts at full p-state
        warm(12)
        pe_transpose(0)
        psT.release()
        psW.release()

        # ---- Phase E: o1T[d] = sum_kt xp[kt,d]^T @ attn^T[kt] ------------
        wovstream = tc.alloc_tile_pool(name="wovstream", bufs=2)
        xpstream2 = tc.alloc_tile_pool(name="xpstream2", bufs=2)
        with tc.tile_pool(name="psE", bufs=1, space="PSUM") as psE_pool:
            psE = [
                psE_pool.tile([128, RQ], f32, name=f"psE{d}") for d in range(KC)
            ]
            # Pass 1: kts 8..31 then 0..7, slots 1-3 only — their exps all
            # finish during B (B runs slots largest-first), so E starts
            # right after B. Pass 2: the slot0-only matmuls for kts 0..7
            # run last, by which time slot0's post-B exp/transpose has
            # landed; its xp chunks (loaded last) are still resident.
            # xp is loaded in 4-kt superchunks to stay under the DMA
            # ring-depth throttle.
            kt_chunks = [24, 28, 16, 20, 8, 12, 0, 4]
            for ci, kt0 in enumerate(kt_chunks):
                pool = xpstream2 if kt0 < 8 else xpstream
                xp_t = pool.tile([128, 4 * D], f16, name="xp_t", tag="xp")
                nc.sync.dma_start(
                    xp_t.rearrange("p (four n) -> p four n", four=4),
                    xp_d[kt0 * 128 : (kt0 + 4) * 128, :].rearrange(
                        "(four p) n -> p four n", p=128
                    ),
                )
                c = kt0 // 8
                width = (4 - c) * 128
                for j in range(4):
                    kt = kt0 + j
                    off = PTOFF[c] + (kt - 8 * c) * width
                    for d in range(KC):
                        stat = xp_t[:, j * D + d * 128 : j * D + (d + 1) * 128]
                        # start_tensor_calc zeroes the WHOLE psum bank, so
                        # only the first matmul into bank d sets it; later
                        # slot regions accumulate onto zeros. All chains end
                        # in the final kt block (0..7).
                        nc.tensor.matmul(
                            psE[d][:, c * 128 : 512],
                            stat,
                            ptall[:, off : off + width],
                            start=(ci == 0 and j == 0),
                            stop=(kt == 7),
                            skip_group_check=True,
                        )
            # evacuate: split across DVE and Act so phase F starts sooner
            for d in range(KC):
                if d % 2 == 0:
                    nc.vector.tensor_copy(o1t[d][:], psE[d][:])
                else:
                    nc.scalar.activation(
                        o1t[d][:],
                        psE[d][:],
                        mybir.ActivationFunctionType.Copy,
                    )

        # ---- Phase F: out = (o1 @ Wov) * recip ---------------------------
        with (
            tc.tile_pool(name="psF", bufs=2, space="PSUM") as psF,
            tc.tile_pool(name="outp", bufs=3) as outp,
        ):
            for nb in range(2):
                wov_blk = wovstream.tile(
                    [128, KC * 512], f16, name="wov_blk", tag="wv"
                )
                nc.sync.dma_start(
                    wov_blk.rearrange("p (kc n) -> p kc n", kc=KC),
                    wov_d[:, nb * 512 : (nb + 1) * 512].rearrange(
                        "(kc p) n -> p kc n", p=128
                    ),
                )
                for m in range(4):
                    ps = psF.tile([128, 512], f32, name="ps_o")
                    for kc in range(KC):
                        nc.tensor.matmul(
                            ps[:],
                            o1t[kc][:, m * 128 : (m + 1) * 128],
                            wov_blk[:, kc * 512 : (kc + 1) * 512],
                            start=(kc == 0),
                            stop=(kc == KC - 1),
                        )
                    ob = outp.tile([128, 512], f32, name="ob")
                    nc.vector.tensor_scalar_mul(ob[:], ps[:], recip[:, m : m + 1])
                    nc.sync.dma_start(
                        out_d[m * 128 : (m + 1) * 128, nb * 512 : (nb + 1) * 512],
                        ob[:],
                    )

        xpstream2.release()
        wovstream.release()
        p_pool.release()
        xpstream.release()
        o1_pool.release()
        pt_pool.release()
        consts.release()

    nc.compile()
    return nc


_NC_CACHE = {}


def _get_nc():
    if "nc" not in _NC_CACHE:
        _NC_CACHE["nc"] = _build_nc()
    return _NC_CACHE["nc"]


def _slot_tiles(c):
    return [c, 15 - c, 16 + c, 31 - c]


def _prep_in_maps(x, Wqk, Wov):
    x = np.ascontiguousarray(np.asarray(x), dtype=np.float32)
    Wqk = np.ascontiguousarray(np.asarray(Wqk), dtype=np.float32)
    Wov = np.ascontiguousarray(np.asarray(Wov), dtype=np.float32)
    x16 = x.astype(np.float16)
    xT16 = np.ascontiguousarray(x16.T)  # [D, T]
    wqk16 = Wqk.astype(np.float16)
    wov16 = Wov.astype(np.float16)

    in_maps = []
    for c in range(NCORES):
        tiles = _slot_tiles(c)
        rows = np.concatenate(
            [np.arange(t * 128, (t + 1) * 128) for t in tiles]
        )
        xqt = np.ascontiguousarray(xT16[:, rows])
        import ml_dtypes

        mask = np.full((128, STOT), NEG16, dtype=ml_dtypes.float8_e5m2)
        p = np.arange(128)[:, None]
        for m, t in enumerate(tiles):
            g = t * 128 + p  # global row index per partition
            y = np.arange(BKT[m] * 128)[None, :]  # global key index
            mask[:, OFFK[m] : OFFK[m] + BKT[m] * 128] = np.where(
                y <= g, 0.0, NEG16
            ).astype(ml_dtypes.float8_e5m2)
        in_maps.append(
            {
                "xqt": xqt,
                "xtp": xT16,
                "xp": x16,
                "wqk": wqk16,
                "wov": wov16,
                "mask": mask,
                "ident": np.eye(128, dtype=np.float16),
            }
        )
    return in_maps


def run(x, Wqk, Wov, **spmd_kwargs):
    """Full pipeline; returns (output [T, D] fp32, BassKernelResults)."""
    import time

    nc = _get_nc()
    in_maps = _prep_in_maps(x, Wqk, Wov)
    try:
        res = run_bass_kernel_spmd(
            nc, in_maps, core_ids=list(range(NCORES)), **spmd_kwargs
        )
    except Exception:
        # a prior crashed execution can leave a core transiently
        # unrecoverable; the runtime resets it — retry once
        time.sleep(10)
        res = run_bass_kernel_spmd(
            nc, in_maps, core_ids=list(range(NCORES)), **spmd_kwargs
        )
    out = np.empty((T, D), dtype=np.float32)
    for c in range(NCORES):
        co = res.results[c]["out"]
        for m, t in enumerate(_slot_tiles(c)):
            out[t * 128 : (t + 1) * 128] = co[m * 128 : (m + 1) * 128]
    return np.ascontiguousarray(out), res


def kernel(x, Wqk, Wov):
    out, _ = run(x, Wqk, Wov)
    return out
